# revision 5
# baseline (speedup 1.0000x reference)
"""GraphConv + BatchNorm + LeakyReLU fused layer on 8 Trainium2 NeuronCores.

Strategy (v2 — dense fp8 edge stream, cell-packed segment sum):
  - Destination nodes are degree-balanced across the 8 cores (snake deal of
    the degree-sorted node list), then packed into "cells" of up to 4 dst
    nodes whose in-edges total <= 128 (fold packing + swap repair). Each
    cell's gathered source features form one [128 edge x 128 feat] fp8
    (e3m4) tile; the host materializes the whole per-core stream densely in
    the exact SBUF layout, so the device streams it at full HBM bandwidth
    with zero SWDGE descriptor cost.
  - The per-cell segment sum is one PE matmul with a [128 x 4] one-hot
    S tile: aggT[:, 4c:4c+4] = G_cell^T @ S_cell. Output free dim is 4, so
    the whole aggregation costs ~4 PE cycles per cell. Cells' PSUM column
    windows are disjoint, so every matmul is start=True/stop=True.
  - Groups of 128 cells (512 dst columns = one PSUM bank) pipeline:
    aggT -> bf16, x1T = WrT.T@aggT + WoT.T@xoT, leaky_relu folded into the
    next matmul (x3T = (0.01 Wl)T.T@(x1+b) + (0.99 Wl)T.T@relu(x1+b)),
    x3 -> bf16 SBUF, per-group BN partial stats (sum via DVE reduce, sum of
    squares via ACT Square accum).
  - Pad dst columns all carry the constant x3 = W_lin @ leaky(b_rel); the
    device computes that constant and subtracts n_pad * c (and n_pad * c^2)
    from the stats before the AllReduce, so statistics are exact over the
    50000 real nodes.
  - BN stats AllReduce via a DRAM bounce; final affine + leaky runs batched
    on the [feat x node] activations in two halves (ACT/DVE/DMA overlap) and
    is stored feature-major as bf16; the host transposes/unpermutes.

kernel(**inputs) takes the full-size numpy inputs and returns the full
[50000, 128] float32 output; everything device-side runs SPMD on cores 0-7.
"""
import sys

if "/opt/trn_rl_repo" not in sys.path:
    sys.path.insert(0, "/opt/trn_rl_repo")

import numpy as np
import ml_dtypes

import concourse.bass as bass
import concourse.mybir as mybir
import concourse.tile as tile
from concourse import bacc
from concourse import bass_utils

F32 = mybir.dt.float32
BF16 = mybir.dt.bfloat16
F8 = mybir.dt.float8e3

N_NODES = 50000
N_CORES = 8
NPC = N_NODES // N_CORES          # 6250 real dst nodes per core
BN_EPS = 1e-5
NEG = 0.01


def _pack_cells(nodes, deg, T, max_iter=4000):
    """Pack `nodes` (approx sorted desc by degree) into T cells of <=4 nodes
    with per-cell degree sum <= 128. Fold packing + swap repair. Returns
    [T, 4] node ids (-1 = empty slot) or None if infeasible."""
    n = len(nodes)
    a = np.full(4 * T, -1, np.int64)
    a[:n] = nodes
    idx = np.arange(T)
    cells = np.stack([a[idx], a[2 * T - 1 - idx], a[2 * T + idx],
                      a[4 * T - 1 - idx]], 1)
    cdeg = np.where(cells >= 0, deg[np.maximum(cells, 0)], 0)
    s = cdeg.sum(1)
    for _ in range(max_iter):
        mx = s.max()
        if mx <= 128:
            return cells
        hi = int(np.argmax(s))
        over = mx - 128
        done = False
        for j in np.argsort(-cdeg[hi]):
            if cells[hi, j] < 0:
                continue
            dj = cdeg[hi, j]
            hi_dk = dj - over
            if hi_dk < 0:
                continue
            lo_dk = np.maximum(s + dj - 128, 0)
            ok = (cdeg >= lo_dk[:, None]) & (cdeg <= hi_dk) & (cells >= 0)
            ok[hi] = False
            tt, kk = np.nonzero(ok)
            if len(tt) == 0:
                continue
            b = int(np.argmax(cdeg[tt, kk]))
            t, k = int(tt[b]), int(kk[b])
            dk = cdeg[t, k]
            cells[hi, j], cells[t, k] = cells[t, k], cells[hi, j]
            cdeg[hi, j], cdeg[t, k] = dk, dj
            s[hi] += dk - dj
            s[t] += dj - dk
            done = True
            break
        if not done:
            return None
    return None


def preprocess(x, edge_index, cfg):
    """Host-side sharding: per-core input dicts (without weights). Sets
    cfg['T'] (cells per core), cfg['n_pad'], and cfg['colmap'] (per-core
    (node ids, device columns) for output unpermutation)."""
    ncores = cfg["n_cores"]
    n = x.shape[0]
    src = np.asarray(edge_index[0], dtype=np.int64)
    dst = np.asarray(edge_index[1], dtype=np.int64)

    deg = np.bincount(dst, minlength=n)
    order = np.argsort(-deg, kind="stable")
    grid = order.reshape(n // ncores, ncores).copy()
    grid[1::2] = grid[1::2, ::-1]  # snake deal: balances per-core edges
    core_nodes = [grid[:, c] for c in range(ncores)]

    T = (n // ncores + 3) // 4
    cells_per_core = None
    while True:
        res = [_pack_cells(cn, deg, T) for cn in core_nodes]
        if all(r is not None for r in res):
            cells_per_core = res
            break
        T += 4
        assert T < 2200, "cell packing runaway"
    cfg["T"] = T
    cfg["n_pad"] = 4 * T - n // ncores

    # node -> (core, cell, pos)
    node_core = np.empty(n, np.int64)
    node_cell = np.empty(n, np.int64)
    node_pos = np.empty(n, np.int64)
    for c in range(ncores):
        cells = cells_per_core[c]
        t_idx, j_idx = np.nonzero(cells >= 0)
        nid = cells[t_idx, j_idx]
        node_core[nid] = c
        node_cell[nid] = t_idx
        node_pos[nid] = j_idx

    # edge -> (core, cell, pos, rank-within-cell)
    ec = node_core[dst]
    et = node_cell[dst]
    ep = node_pos[dst]
    key = ec * T + et
    eorder = np.argsort(key, kind="stable")
    key_s = key[eorder]
    counts = np.bincount(key_s, minlength=ncores * T)
    starts = np.zeros(ncores * T + 1, np.int64)
    np.cumsum(counts, out=starts[1:])
    rank_s = np.arange(len(src)) - starts[key_s]
    assert rank_s.max() < 128
    src_s = src[eorder]
    ec_s = ec[eorder]
    et_s = et[eorder]
    ep_s = ep[eorder]

    xq = x.astype(ml_dtypes.float8_e3m4)
    xb = x.astype(ml_dtypes.bfloat16)

    G_all = np.zeros((ncores, T, 128, 128), dtype=ml_dtypes.float8_e3m4)
    G_all[ec_s, et_s, rank_s] = xq[src_s]
    S_all = np.zeros((ncores, 128, T, 4), dtype=ml_dtypes.float8_e3m4)
    S_all[ec_s, rank_s, et_s, ep_s] = 1.0

    per_core = []
    colmap = []
    for c in range(ncores):
        cells = cells_per_core[c]
        t_idx, j_idx = np.nonzero(cells >= 0)
        nid = cells[t_idx, j_idx]
        cols = 4 * t_idx + j_idx
        xoT = np.zeros((128, 4 * T), dtype=ml_dtypes.bfloat16)
        xoT[:, cols] = xb[nid].T
        per_core.append({
            "G": np.ascontiguousarray(G_all[c].transpose(1, 0, 2)),
            "S": np.ascontiguousarray(S_all[c]),
            "xoT": xoT,
        })
        colmap.append((nid, cols))
    cfg["colmap"] = colmap
    return per_core


def build_program(cfg):
    ncores = cfg["n_cores"]
    T = cfg["T"]
    n_pad = cfg["n_pad"]
    W = 4 * T
    groups = []
    c0 = 0
    while c0 < T:
        groups.append((c0, min(128, T - c0)))
        c0 += 128
    ng = len(groups)
    inv_n = 1.0 / float(cfg["n_total"])

    nc = bacc.Bacc("TRN2", target_bir_lowering=False, debug=False,
                   num_devices=ncores)

    G_d = nc.dram_tensor("G", [128, T, 128], F8, kind="ExternalInput")
    S_d = nc.dram_tensor("S", [128, T, 4], F8, kind="ExternalInput")
    xo_d = nc.dram_tensor("xoT", [128, W], BF16, kind="ExternalInput")
    wr_d = nc.dram_tensor("WrT", [128, 128], BF16, kind="ExternalInput")
    wo_d = nc.dram_tensor("WoT", [128, 128], BF16, kind="ExternalInput")
    wl_d = nc.dram_tensor("WlT", [128, 128], BF16, kind="ExternalInput")
    br_d = nc.dram_tensor("brel", [128, 1], F32, kind="ExternalInput")
    ga_d = nc.dram_tensor("gamma", [128, 1], F32, kind="ExternalInput")
    be_d = nc.dram_tensor("beta", [128, 1], F32, kind="ExternalInput")
    out_d = nc.dram_tensor("out", [128, W], BF16, kind="ExternalOutput")

    AF = mybir.ActivationFunctionType
    with tile.TileContext(nc) as tc:
        with (
            tc.tile_pool(name="consts", bufs=1) as consts,
            tc.tile_pool(name="gp", bufs=3) as gp,
            tc.tile_pool(name="sp", bufs=3) as sp,
            tc.tile_pool(name="ps", bufs=6, space="PSUM") as ps,
            tc.tile_pool(name="misc", bufs=3) as misc,
            tc.tile_pool(name="big", bufs=1) as big,
            tc.tile_pool(name="dram", bufs=1, space="DRAM") as dram,
        ):
            wr_s = consts.tile([128, 128], BF16)
            wo_s = consts.tile([128, 128], BF16)
            wl_s = consts.tile([128, 128], BF16)
            br_s = consts.tile([128, 1], F32)
            ga_s = consts.tile([128, 1], F32)
            be_s = consts.tile([128, 1], F32)
            xot_s = big.tile([128, W], BF16)
            x3_s = big.tile([128, W], BF16)
            out_sb = big.tile([128, W], BF16)
            sums = big.tile([128, ng], F32)
            sqs = big.tile([128, ng], F32)
            junk = big.tile([128, 512], F32)

            nc.gpsimd.dma_start(wr_s[:], wr_d[:])
            nc.gpsimd.dma_start(wo_s[:], wo_d[:])
            nc.gpsimd.dma_start(wl_s[:], wl_d[:])
            nc.gpsimd.dma_start(br_s[:], br_d[:])
            nc.gpsimd.dma_start(ga_s[:], ga_d[:])
            nc.gpsimd.dma_start(be_s[:], be_d[:])

            # c* = W_lin @ leaky(b_rel): the x3 value of every pad column.
            zero1 = consts.tile([128, 1], F32)
            nc.vector.memset(zero1[:], 0.0)
            vb = consts.tile([128, 1], BF16)
            nc.scalar.activation(vb[:], br_s[:], AF.Lrelu, bias=zero1[:],
                                 scale=1.0, alpha=NEG)
            cst_ps = ps.tile([128, 1], F32, tag="ps")
            nc.tensor.matmul(cst_ps[:], lhsT=wl_s[:], rhs=vb[:],
                             start=True, stop=True)
            cst = consts.tile([128, 1], F32)
            cst2 = consts.tile([128, 1], F32)
            nc.scalar.copy(cst[:], cst_ps[:])
            nc.vector.tensor_tensor(out=cst2[:], in0=cst[:], in1=cst[:],
                                    op=mybir.AluOpType.mult)

            for g, (c0, cg) in enumerate(groups):
                Gt = gp.tile([128, 128, 128], F8, tag="G")
                St = sp.tile([128, 128, 4], F8, tag="S")
                xot_c = sp.tile([128, 512], BF16, tag="xo")
                nc.gpsimd.dma_start(Gt[:, 0:cg, :], G_d[:, c0:c0 + cg, :])
                nc.gpsimd.dma_start(St[:, 0:cg, :], S_d[:, c0:c0 + cg, :])
                nc.gpsimd.dma_start(xot_c[:, 0:cg * 4],
                                    xo_d[:, 4 * c0:4 * (c0 + cg)])
                agg_ps = ps.tile([128, 128, 4], F32, tag="ps")
                for i in range(cg):
                    nc.tensor.matmul(agg_ps[:, i, :], lhsT=Gt[:, i, :],
                                     rhs=St[:, i, :], start=True, stop=True)
                aggs = misc.tile([128, 512], BF16, tag="aggs")
                nc.scalar.copy(aggs[:, 0:cg * 4], agg_ps[:, 0:cg, :])

                x1_ps = ps.tile([128, 512], F32, tag="ps")
                nc.tensor.matmul(x1_ps[:, 0:cg * 4], lhsT=wr_s[:],
                                 rhs=aggs[:, 0:cg * 4], start=True,
                                 stop=False)
                nc.tensor.matmul(x1_ps[:, 0:cg * 4], lhsT=wo_s[:],
                                 rhs=xot_c[:, 0:cg * 4],
                                 start=False, stop=True)
                v_t = misc.tile([128, 512], BF16, tag="v")
                nc.scalar.activation(v_t[:, 0:cg * 4], x1_ps[:, 0:cg * 4],
                                     AF.Lrelu, bias=br_s[:], scale=1.0,
                                     alpha=NEG)
                x3_ps = ps.tile([128, 512], F32, tag="ps")
                nc.tensor.matmul(x3_ps[:, 0:cg * 4], lhsT=wl_s[:],
                                 rhs=v_t[:, 0:cg * 4], start=True, stop=True)
                xr = x3_s[:, 4 * c0:4 * (c0 + cg)]
                nc.vector.tensor_copy(xr, x3_ps[:, 0:cg * 4])
                nc.vector.tensor_reduce(sums[:, g:g + 1], xr,
                                        axis=mybir.AxisListType.X,
                                        op=mybir.AluOpType.add)
                nc.scalar.activation(junk[:, 0:cg * 4], xr, AF.Square,
                                     accum_out=sqs[:, g:g + 1])

            # ---- global BN statistics (pad-corrected) via AllReduce ----
            sumt = consts.tile([128, 1], F32)
            sqt = consts.tile([128, 1], F32)
            stat2 = consts.tile([128, 2], F32)
            nc.vector.tensor_reduce(sumt[:], sums[:],
                                    axis=mybir.AxisListType.X,
                                    op=mybir.AluOpType.add)
            nc.vector.tensor_reduce(sqt[:], sqs[:],
                                    axis=mybir.AxisListType.X,
                                    op=mybir.AluOpType.add)
            nc.vector.scalar_tensor_tensor(
                out=stat2[:, 0:1], in0=cst[:], scalar=-float(n_pad),
                in1=sumt[:], op0=mybir.AluOpType.mult,
                op1=mybir.AluOpType.add)
            nc.vector.scalar_tensor_tensor(
                out=stat2[:, 1:2], in0=cst2[:], scalar=-float(n_pad),
                in1=sqt[:], op0=mybir.AluOpType.mult,
                op1=mybir.AluOpType.add)

            cc_in = dram.tile([128, 2], F32)
            cc_out = dram.tile([128, 2], F32)
            nc.gpsimd.dma_start(cc_in[:], stat2[:])
            if ncores > 1 and not cfg.get("no_cc"):
                nc.gpsimd.collective_compute(
                    "AllReduce",
                    mybir.AluOpType.add,
                    replica_groups=[list(range(ncores))],
                    ins=[cc_in[:].opt()],
                    outs=[cc_out[:].opt()],
                )
                red = cc_out
            else:
                red = cc_in
            stat_r = consts.tile([128, 2], F32)
            nc.sync.dma_start(stat_r[:], red[:])

            mean = consts.tile([128, 1], F32)
            ex2 = consts.tile([128, 1], F32)
            var = consts.tile([128, 1], F32)
            rstd = consts.tile([128, 1], F32)
            scl = consts.tile([128, 1], F32)
            bia = consts.tile([128, 1], F32)
            tmp1 = consts.tile([128, 1], F32)
            nc.vector.tensor_scalar_mul(mean[:], stat_r[:, 0:1], inv_n)
            nc.vector.tensor_scalar_mul(ex2[:], stat_r[:, 1:2], inv_n)
            nc.vector.tensor_tensor(out=tmp1[:], in0=mean[:], in1=mean[:],
                                    op=mybir.AluOpType.mult)
            nc.vector.tensor_sub(var[:], ex2[:], tmp1[:])
            epsv = consts.tile([128, 1], F32)
            nc.vector.memset(epsv[:], BN_EPS)
            nc.scalar.activation(rstd[:], var[:], AF.Sqrt, bias=epsv[:],
                                 scale=1.0)
            nc.vector.reciprocal(rstd[:], rstd[:])
            nc.vector.tensor_tensor(out=scl[:], in0=ga_s[:], in1=rstd[:],
                                    op=mybir.AluOpType.mult)
            nc.vector.tensor_tensor(out=tmp1[:], in0=mean[:], in1=scl[:],
                                    op=mybir.AluOpType.mult)
            nc.vector.tensor_sub(bia[:], be_s[:], tmp1[:])

            # ---- normalize + leaky + store, in halves for overlap ----
            half = (W // 2 + 3) & ~3
            for h0, h1 in ((0, half), (half, W)):
                nc.scalar.activation(x3_s[:, h0:h1], x3_s[:, h0:h1],
                                     AF.Identity, bias=bia[:], scale=scl[:])
                nc.vector.scalar_tensor_tensor(
                    out=out_sb[:, h0:h1], in0=x3_s[:, h0:h1], scalar=NEG,
                    in1=x3_s[:, h0:h1], op0=mybir.AluOpType.mult,
                    op1=mybir.AluOpType.max)
                nc.sync.dma_start(out_d[:, h0:h1], out_sb[:, h0:h1])

    nc.compile()
    return nc


_PROGRAM_CACHE = {}


def run(x, edge_index, W_rel, b_rel, W_root, W_lin, b_lin, gamma, beta, cfg):
    per_core = preprocess(x, edge_index, cfg)

    shared = {
        "WrT": np.ascontiguousarray(W_rel.T).astype(ml_dtypes.bfloat16),
        "WoT": np.ascontiguousarray(W_root.T).astype(ml_dtypes.bfloat16),
        "WaT": np.ascontiguousarray((NEG * W_lin).T).astype(
            ml_dtypes.bfloat16),
        "WbT": np.ascontiguousarray(((1.0 - NEG) * W_lin).T).astype(
            ml_dtypes.bfloat16),
        "brel": b_rel.reshape(128, 1).astype(np.float32),
        "gamma": gamma.reshape(128, 1).astype(np.float32),
        "beta": beta.reshape(128, 1).astype(np.float32),
    }
    # b_lin is dropped: it shifts every x3 column equally, so BatchNorm's
    # mean subtraction cancels it exactly.
    in_maps = [dict(m, **shared) for m in per_core]

    key = (cfg["n_cores"], cfg["T"])
    if key not in _PROGRAM_CACHE:
        _PROGRAM_CACHE[key] = build_program(cfg)
    nc = _PROGRAM_CACHE[key]

    res = bass_utils.run_bass_kernel_spmd(
        nc, in_maps, core_ids=list(range(cfg["n_cores"])))
    n = x.shape[0]
    out = np.empty((n, 128), dtype=np.float32)
    for c in range(cfg["n_cores"]):
        nid, cols = cfg["colmap"][c]
        dev = np.asarray(res.results[c]["out"])  # [128, 4T] bf16
        out[nid] = dev[:, cols].T.astype(np.float32)
    return out


def make_cfg():
    return {
        "n_cores": N_CORES,
        "npc": NPC,
        "n_total": N_NODES,
    }


def kernel(x, edge_index, batch, W_rel, b_rel, W_root, W_lin, b_lin, gamma,
           beta):
    x = np.asarray(x, dtype=np.float32)
    cfg = make_cfg()
    return run(x, np.asarray(edge_index), np.asarray(W_rel, dtype=np.float32),
               np.asarray(b_rel, dtype=np.float32),
               np.asarray(W_root, dtype=np.float32),
               np.asarray(W_lin, dtype=np.float32),
               np.asarray(b_lin, dtype=np.float32),
               np.asarray(gamma, dtype=np.float32),
               np.asarray(beta, dtype=np.float32), cfg)
